# revision 51
# baseline (speedup 1.0000x reference)
"""AIFI block (linear attention + dwconv + FFN) on 8 TRN2 NeuronCores.

Data-parallel over batch: core i computes batch element i entirely on-core.
Feature-major [C, N] activation layout (x's natural layout) so no input or
output transposes are needed. Matmuls in bf16, residual stream in fp32.

Work split across engines (per 512-col chunk):
  PE   : all matmuls + depthwise center tap (PSUM init)
  DVE  : k/v bias drain, 5 depthwise taps (PSUM RMW), dw merge, repbn folds
  ACT  : q relu drain, attn drain, gelu, u12 = A1A2*x + u12b
  GPS  : xb DMA-cast, k relu, 3 depthwise taps into bf16 accumulator

RepBN eval folds (host-side): t1 = A1*(x + proj_raw) + B1', with
v = A2*t1 + B2' computed directly as v = A1A2*proj_psum + u12,
t1_bf16 = v*invA2 + negB2oA2, out = A2*fc2_psum + v.
"""

import sys

import numpy as np

_REPO = "/opt/trn_rl_repo"
if _REPO not in sys.path:
    sys.path.insert(0, _REPO)

B, C, HH, WW = 8, 256, 64, 64
N = HH * WW  # 4096 tokens
NH, HD = 8, 32
CM = 2048
EPS = 1e-5
SCALE = HD ** -0.5
P = 128
NTC = 512          # columns per n-tile
NT = N // NTC      # 8 n-tiles
TT = N // P        # 32 token tiles
MH = CM // P       # 16 hidden chunks
YB = NTC // WW     # 8 y-rows per n-tile

_CACHE = {}

# tap -> engine: PE takes the dy=0 row (3 diag matmuls); DVE the dy=+-1 rows
DVE_TAPS = [(-1, -1), (-1, 0), (-1, 1), (1, -1), (1, 0), (1, 1)]


def _build_nc(reps=1):
    import concourse.bass as bass
    import concourse.tile as tile
    from concourse import bacc, mybir
    from concourse.masks import make_identity

    f32 = mybir.dt.float32
    bf16 = mybir.dt.bfloat16
    Relu = mybir.ActivationFunctionType.Relu
    Gelu = mybir.ActivationFunctionType.Gelu
    Copy = mybir.ActivationFunctionType.Copy
    Ident = mybir.ActivationFunctionType.Identity
    add = mybir.AluOpType.add
    mult = mybir.AluOpType.mult

    nc = bacc.Bacc(None, target_bir_lowering=False)

    x_ext = nc.declare_dram_parameter("x", [C, HH, WW], bf16, isOutput=False)
    wqkv_ext = nc.declare_dram_parameter("wqkv", [C, 3 * C], bf16, isOutput=False)
    wproj_ext = nc.declare_dram_parameter("wproj", [C, C], bf16, isOutput=False)
    wfc1_ext = nc.declare_dram_parameter("wfc1", [C, CM], bf16, isOutput=False)
    wfc2_ext = nc.declare_dram_parameter("wfc2", [CM, C], bf16, isOutput=False)
    dww_ext = nc.declare_dram_parameter("dww", [C, 9], f32, isOutput=False)
    # pcst columns: 0=bq 1=dwb 2=A1A2 3=u12b 4=A2 5=invA2 6=negB2oA2 7=pad
    pcst_ext = nc.declare_dram_parameter("pcst", [C, 8], f32, isOutput=False)
    bkv_ext = nc.declare_dram_parameter("bkv", [2 * C], f32, isOutput=False)
    bfc1_ext = nc.declare_dram_parameter("bfc1c", [P, MH], f32, isOutput=False)
    out_ext = nc.declare_dram_parameter("out", [C, HH, WW], f32, isOutput=True)

    def bcast(ap_1d, parts=P):
        """[n] dram AP -> [parts, n] AP with 0-stride partition dim."""
        return bass.AP(
            tensor=ap_1d.tensor,
            offset=ap_1d.offset,
            ap=[[0, parts]] + list(ap_1d.ap),
        )

    with tile.TileContext(nc) as tc:
        with (
            tc.tile_pool(name="persist", bufs=1) as persist,
            tc.tile_pool(name="small", bufs=1) as small,
            tc.tile_pool(name="bigbf", bufs=6) as bigbf,
            tc.tile_pool(name="kvpool", bufs=4) as kvpool,
            tc.tile_pool(name="hpool", bufs=3) as hpool,
            tc.tile_pool(name="dwsb", bufs=6) as dwsb_pool,
            tc.tile_pool(name="outsb", bufs=2) as outsb_pool,
            tc.tile_pool(name="psA", bufs=4, space="PSUM") as psA,
            tc.tile_pool(name="psDw", bufs=2, space="PSUM") as psDw,
            tc.tile_pool(name="psHold", bufs=2, space="PSUM") as psHold,
        ):
            for rep in range(reps):
                # ------------- load x (bf16, cast host-side) --------------
                x2d = x_ext[:].rearrange("c h w -> c (h w)")
                xb = [bigbf.tile([P, N], bf16, tag="bigbf", name="xb")
                      for _ in range(2)]
                wqkv = [persist.tile([P, 3 * C], bf16, tag=f"wqkv_{k}",
                                     name="wqkv") for k in range(2)]
                wproj = [persist.tile([P, C], bf16, tag=f"wproj_{k}",
                                      name="wproj") for k in range(2)]
                wfc1 = [persist.tile([P, CM], bf16, tag=f"wfc1_{k}",
                                     name="wfc1") for k in range(2)]
                wfc2 = persist.tile([P, MH, C], bf16, tag="wfc2", name="wfc2")
                # ------------- small constants ----------------------------
                pcst = [small.tile([P, 8], f32, tag=f"pcst_{m}", name="pcst")
                        for m in range(2)]
                for m in range(2):
                    nc.gpsimd.dma_start(
                        out=pcst[m][:], in_=pcst_ext[m * P:(m + 1) * P, :]
                    )
                bq = [pcst[m][:, 0:1] for m in range(2)]
                dwb = [pcst[m][:, 1:2] for m in range(2)]
                A1A2 = [pcst[m][:, 2:3] for m in range(2)]
                u12b = [pcst[m][:, 3:4] for m in range(2)]
                A2 = [pcst[m][:, 4:5] for m in range(2)]
                invA2 = [pcst[m][:, 5:6] for m in range(2)]
                negB2oA2 = [pcst[m][:, 6:7] for m in range(2)]

                bias_kv = small.tile([P, 2 * C], f32, tag="bias_kv",
                                     name="bias_kv")
                nc.gpsimd.dma_start(out=bias_kv[:], in_=bcast(bkv_ext[:]))

                bfc1_sb = small.tile([P, MH], f32, tag="bfc1_sb",
                                     name="bfc1_sb")
                nc.gpsimd.dma_start(out=bfc1_sb[:], in_=bfc1_ext[:])

                dma_engs = [nc.sync, nc.scalar, nc.sync, nc.scalar]
                for k in range(2):
                    dma_engs[k].dma_start(
                        out=wqkv[k][:], in_=wqkv_ext[k * P:(k + 1) * P, :]
                    )
                for j in range(4):
                    s = slice(j * 1024, (j + 1) * 1024)
                    for m in range(2):
                        rows = slice(m * P, (m + 1) * P)
                        dma_engs[(2 * j + m) % 4].dma_start(
                            out=xb[m][:, s], in_=x2d[rows, s]
                        )
                for k in range(2):
                    rows = slice(k * P, (k + 1) * P)
                    nc.gpsimd.dma_start(out=wproj[k][:], in_=wproj_ext[rows, :])
                    dma_engs[k].dma_start(out=wfc1[k][:], in_=wfc1_ext[rows, :])
                for k in range(4):
                    dma_engs[k % 2].dma_start(
                        out=wfc2[:, 4 * k:4 * (k + 1), :],
                        in_=wfc2_ext[:].rearrange("(kt p) c -> p kt c", p=P)[
                            :, 4 * k:4 * (k + 1), :
                        ],
                    )


                # kv allocated first so v_sb (phase 3) reuses its slots
                kv_sb = [kvpool.tile([P, TT // 2, 2 * C], bf16, tag="kvpool",
                                     name="kv_sb") for _ in range(2)]
                u12 = [kvpool.tile([P, N], f32, tag="kvpool", name="u12")
                       for _ in range(2)]
                for m in range(2):
                    for j in range(4):
                        s = slice(j * 1024, (j + 1) * 1024)
                        nc.scalar.activation(
                            u12[m][:, s], xb[m][:, s], Ident,
                            bias=u12b[m], scale=A1A2[m],
                        )
                # ------------- phase 1: k,v then q then ctx ---------------
                q_sb = [bigbf.tile([P, N], bf16, tag="bigbf", name="q_sb")
                        for _ in range(2)]
                ctx_ps = [psDw.tile([P, NTC], f32, tag="psDw",
                                    name="ctx_ps") for _ in range(2)]

                def kv_step(tt):
                    ti, j = divmod(tt, TT // 2)
                    tcols = slice(tt * P, (tt + 1) * P)
                    ps = psA.tile([P, 2 * C], f32, tag="psA", name="kv_ps")
                    for k in range(2):
                        nc.tensor.matmul(
                            ps[:],
                            xb[k][:, tcols],
                            wqkv[k][:, C:3 * C],
                            start=(k == 0),
                            stop=(k == 1),
                        )
                    sl = kv_sb[ti][:, j, :]
                    nc.vector.tensor_tensor(sl, ps[:], bias_kv[:], add)
                    nc.scalar.activation(
                        kv_sb[ti][:, j, 0:C], kv_sb[ti][:, j, 0:C], Relu,
                        bias=0.0, scale=1.0,
                    )

                def ctx_step(tt):
                    ti, j = divmod(tt, TT // 2)
                    for m in range(2):
                        nc.tensor.matmul(
                            ctx_ps[m][:, 0:C],
                            kv_sb[ti][:, j, m * P:(m + 1) * P],
                            kv_sb[ti][:, j, C:2 * C],
                            start=(tt == 0),
                            stop=(tt == TT - 1),
                        )

                def q_step(nt):
                    cols = slice(nt * NTC, (nt + 1) * NTC)
                    for m in range(2):
                        ps = psA.tile([P, NTC], f32, tag="psA", name="q_ps")
                        for k in range(2):
                            nc.tensor.matmul(
                                ps[:],
                                wqkv[k][:, m * P:(m + 1) * P],
                                xb[k][:, cols],
                                start=(k == 0),
                                stop=(k == 1),
                            )
                        nc.scalar.activation(
                            q_sb[m][:, cols], ps[:], Relu, bias=bq[m],
                            scale=1.0,
                        )

                for tt in range(TT):
                    kv_step(tt)
                    if tt % 4 == 3:
                        q_step(tt // 4)

                ident = small.tile([P, P], bf16, tag="ident", name="ident")
                make_identity(nc, ident[:])
                dwwt = []
                diag_c = []
                for m in range(2):
                    t = small.tile([P, 9], f32, tag=f"dww_{m}", name="dwwt")
                    nc.gpsimd.dma_start(
                        out=t[:], in_=dww_ext[m * P:(m + 1) * P, :]
                    )
                    dwwt.append(t)
                    d = small.tile([P, 9, P], bf16, tag=f"diagc_{m}",
                                   name="diagc")
                    for i in range(9):
                        nc.vector.tensor_scalar_mul(
                            d[:, i, :], ident[:], t[:, i:i + 1]
                        )
                    diag_c.append(d)
                for tt in range(TT):
                    ctx_step(tt)


                # extract per-head diag blocks (scaled) to bf16
                ctx_sb = [small.tile([P, HD], bf16, tag=f"ctx_{m}", name="ctx")
                          for m in range(2)]
                for h in range(NH):
                    m, r = divmod(h, 4)
                    rows = slice(32 * r, 32 * r + 32)
                    nc.scalar.activation(
                        ctx_sb[m][rows, :],
                        ctx_ps[m][rows, 32 * h:32 * h + 32],
                        Copy,
                        scale=SCALE,
                    )

                # ------------- phase 2: attn out (feature-major) ----------
                attn_sb = [bigbf.tile([P, N], bf16, tag="bigbf", name="attn")
                           for _ in range(2)]

                def attn_step(nt):
                    cols = slice(nt * NTC, (nt + 1) * NTC)
                    for m in range(2):
                        ps = psA.tile([P, NTC], f32, tag="psA", name="attn_ps")
                        for j in range(4):
                            rows = slice(32 * j, 32 * j + 32)
                            nc.tensor.matmul(
                                ps[rows, :],
                                ctx_sb[m][rows, :],
                                q_sb[m][rows, cols],
                                start=True,
                                stop=True,
                                tile_position=(32 * j, 32 * j),
                            )
                        nc.scalar.activation(
                            attn_sb[m][:, cols], ps[:], Copy, scale=1.0
                        )

                # ------------- phases 3+4 interleaved per n-tile ----------
                v_sb = [kvpool.tile([P, N], f32, tag="kvpool", name="v_sb")
                        for _ in range(2)]
                t1_b = [bigbf.tile([P, N], bf16, tag="bigbf", name="t1_b")
                        for _ in range(2)]

                def clip(dy, dx, y0):
                    ys = max(y0, -dy)
                    ye = min(y0 + YB, HH - dy)
                    xs = max(0, -dx)
                    xe = min(WW, WW - dx)
                    return ys, ye, xs, xe

                dw_tiles = {}

                def phase3a(nt, pe_all=False):
                    y0 = nt * YB
                    dw_chunks = []
                    for m in range(2):
                        av = attn_sb[m][:].rearrange("p (y x) -> p y x", x=WW)
                        ps = psDw.tile([P, YB, WW], f32, tag="psDw",
                                       name="dw_ps")
                        nc.tensor.matmul(
                            ps[:], diag_c[m][:, 4, :], av[:, y0:y0 + YB, :],
                            start=True, stop=False, skip_group_check=True,
                        )
                        if pe_all:
                            for i, (dy, dx) in enumerate(DVE_TAPS):
                                ys, ye, xs, xe = clip(dy, dx, y0)
                                ti = (dy + 1) * 3 + (dx + 1)
                                nc.tensor.matmul(
                                    ps[:, ys - y0:ye - y0, xs:xe],
                                    diag_c[m][:, ti, :],
                                    av[:, ys + dy:ye + dy, xs + dx:xe + dx],
                                    start=False, stop=False,
                                    skip_group_check=True,
                                )
                        nc.tensor.matmul(
                            ps[:, :, 1:WW], diag_c[m][:, 3, :],
                            av[:, y0:y0 + YB, 0:WW - 1],
                            start=False, stop=False, skip_group_check=True,
                        )
                        nc.tensor.matmul(
                            ps[:, :, 0:WW - 1], diag_c[m][:, 5, :],
                            av[:, y0:y0 + YB, 1:WW],
                            start=False, stop=True, skip_group_check=True,
                        )
                        dve_taps = [] if pe_all else DVE_TAPS
                        for (dy, dx) in dve_taps:
                            ys, ye, xs, xe = clip(dy, dx, y0)
                            ti = (dy + 1) * 3 + (dx + 1)
                            ym = (ys + ye) // 2
                            for (ya, yb_) in ((ys, ym), (ym, ye)):
                                if ya >= yb_:
                                    continue
                                nc.vector.scalar_tensor_tensor(
                                    ps[:, ya - y0:yb_ - y0, xs:xe],
                                    av[:, ya + dy:yb_ + dy, xs + dx:xe + dx],
                                    dwwt[m][:, ti:ti + 1],
                                    ps[:, ya - y0:yb_ - y0, xs:xe],
                                    mult, add,
                                )
                        dwc = dwsb_pool.tile([P, NTC], bf16, tag="dwsb",
                                             name="dwc")
                        nc.vector.tensor_scalar(
                            dwc[:].rearrange("p (y x) -> p y x", x=WW),
                            ps[:], dwb[m], None, add,
                        )
                        dw_chunks.append(dwc)
                    dw_tiles[nt] = dw_chunks

                def phase3b(nt):
                    cols = slice(nt * NTC, (nt + 1) * NTC)
                    dw_chunks = dw_tiles.pop(nt)
                    for mo in range(2):
                        ps = psA.tile([P, NTC], f32, tag="psA", name="proj_ps")
                        for k in range(2):
                            nc.tensor.matmul(
                                ps[:],
                                wproj[k][:, mo * P:(mo + 1) * P],
                                dw_chunks[k][:],
                                start=(k == 0),
                                stop=(k == 1),
                            )
                        nc.vector.scalar_tensor_tensor(
                            v_sb[mo][:, cols], ps[:], A1A2[mo],
                            u12[mo][:, cols], mult, add,
                        )
                        nc.vector.tensor_scalar(
                            t1_b[mo][:, cols], v_sb[mo][:, cols],
                            invA2[mo], negB2oA2[mo], mult, add,
                        )

                def phase4(nt):
                    cols = slice(nt * NTC, (nt + 1) * NTC)
                    h_sb = hpool.tile([P, MH, NTC], bf16, tag="hpool",
                                      name="h_sb")
                    fc2_ps = [psHold.tile([P, NTC], f32, tag="psHold",
                                          name="fc2_ps") for _ in range(2)]
                    for kt in range(MH):
                        ps = psA.tile([P, NTC], f32, tag="psA", name="fc1_ps")
                        for k in range(2):
                            nc.tensor.matmul(
                                ps[:],
                                wfc1[k][:, kt * P:(kt + 1) * P],
                                t1_b[k][:, cols],
                                start=(k == 0),
                                stop=(k == 1),
                            )
                        nc.scalar.activation(
                            h_sb[:, kt, :], ps[:], Gelu,
                            bias=bfc1_sb[:, kt:kt + 1], scale=1.0,
                        )
                        for mo in range(2):
                            nc.tensor.matmul(
                                fc2_ps[mo][:],
                                wfc2[:, kt, mo * P:(mo + 1) * P],
                                h_sb[:, kt, :],
                                start=(kt == 0),
                                stop=(kt == MH - 1),
                            )
                    for mo in range(2):
                        ot = outsb_pool.tile([P, NTC], f32, tag="outsb",
                                             name="outsb")
                        nc.vector.scalar_tensor_tensor(
                            ot[:], fc2_ps[mo][:], A2[mo], v_sb[mo][:, cols],
                            mult, add,
                        )
                        nc.sync.dma_start(
                            out=out_ext[:].rearrange("c h w -> c (h w)")[
                                mo * P:(mo + 1) * P, cols
                            ],
                            in_=ot[:],
                        )

                attn_step(0)
                attn_step(1)
                attn_step(2)
                phase3a(0, pe_all=True)
                for nt in range(NT):
                    if nt + 3 < NT:
                        attn_step(nt + 3)
                    phase3b(nt)
                    if nt + 1 < NT:
                        phase3a(nt + 1, pe_all=(nt == 0))
                    if nt >= 1:
                        phase4(nt - 1)
                phase4(NT - 1)

    nc.compile()
    return nc


def _get_nc(reps=1):
    key = ("nc", reps)
    if key not in _CACHE:
        _CACHE[key] = _build_nc(reps)
    return _CACHE[key]


def _prep_shared(inputs):
    import ml_dtypes

    bf = ml_dtypes.bfloat16
    f = lambda k: np.asarray(inputs[k], dtype=np.float32)

    rs1 = 1.0 / np.sqrt(f("bn1_v") + EPS)
    gr1 = f("bn1_g") * rs1
    A1 = gr1 + f("alpha1")
    B1 = f("bn1_b") - f("bn1_m") * gr1 + A1 * f("bproj")
    rs2 = 1.0 / np.sqrt(f("bn2_v") + EPS)
    gr2 = f("bn2_g") * rs2
    A2 = gr2 + f("alpha2")
    B2 = f("bn2_b") - f("bn2_m") * gr2 + A2 * f("bfc2")

    A1A2 = A1 * A2
    u12b = A2 * B1 + B2
    invA2 = 1.0 / A2
    negB2oA2 = -B2 / A2
    pad = np.zeros_like(A2)

    pcst = np.stack(
        [f("bqkv")[:C], f("dw_b"), A1A2, u12b, A2, invA2, negB2oA2, pad],
        axis=1,
    )

    return {
        "wqkv": np.ascontiguousarray(f("Wqkv").astype(bf)),
        "wproj": np.ascontiguousarray(f("Wproj").astype(bf)),
        "wfc1": np.ascontiguousarray(f("Wfc1").astype(bf)),
        "wfc2": np.ascontiguousarray(f("Wfc2").astype(bf)),
        "dww": np.ascontiguousarray(f("dw_w").reshape(C, 9)),
        "pcst": np.ascontiguousarray(pcst.astype(np.float32)),
        "bkv": np.ascontiguousarray(f("bqkv")[C:]),
        "bfc1c": np.ascontiguousarray(f("bfc1").reshape(MH, P).T),
    }


def kernel(**inputs):
    import ml_dtypes

    from concourse.bass_utils import run_bass_kernel_spmd

    nc = _get_nc()
    shared = _prep_shared(inputs)
    x = np.ascontiguousarray(
        np.asarray(inputs["x"], dtype=np.float32).astype(ml_dtypes.bfloat16)
    )
    in_maps = [dict(shared, x=x[i]) for i in range(B)]
    res = run_bass_kernel_spmd(nc, in_maps, core_ids=list(range(B)))
    return np.stack([res.results[i]["out"] for i in range(B)], axis=0)


def make_in_maps(inputs):
    import ml_dtypes

    shared = _prep_shared(inputs)
    x = np.ascontiguousarray(
        np.asarray(inputs["x"], dtype=np.float32).astype(ml_dtypes.bfloat16)
    )
    return [dict(shared, x=x[i]) for i in range(B)]


# revision 52
# speedup vs baseline: 1.0097x; 1.0097x over previous
"""AIFI block (linear attention + dwconv + FFN) on 8 TRN2 NeuronCores.

Data-parallel over batch: core i computes batch element i entirely on-core.
Feature-major [C, N] activation layout (x's natural layout) so no input or
output transposes are needed. Matmuls in bf16, residual stream in fp32.

Work split across engines (per 512-col chunk):
  PE   : all matmuls + depthwise center tap (PSUM init)
  DVE  : k/v bias drain, 5 depthwise taps (PSUM RMW), dw merge, repbn folds
  ACT  : q relu drain, attn drain, gelu, u12 = A1A2*x + u12b
  GPS  : xb DMA-cast, k relu, 3 depthwise taps into bf16 accumulator

RepBN eval folds (host-side): t1 = A1*(x + proj_raw) + B1', with
v = A2*t1 + B2' computed directly as v = A1A2*proj_psum + u12,
t1_bf16 = v*invA2 + negB2oA2, out = A2*fc2_psum + v.
"""

import sys

import numpy as np

_REPO = "/opt/trn_rl_repo"
if _REPO not in sys.path:
    sys.path.insert(0, _REPO)

B, C, HH, WW = 8, 256, 64, 64
N = HH * WW  # 4096 tokens
NH, HD = 8, 32
CM = 2048
EPS = 1e-5
SCALE = HD ** -0.5
P = 128
NTC = 512          # columns per n-tile
NT = N // NTC      # 8 n-tiles
TT = N // P        # 32 token tiles
MH = CM // P       # 16 hidden chunks
YB = NTC // WW     # 8 y-rows per n-tile

_CACHE = {}

# tap -> engine: PE takes the dy=0 row (3 diag matmuls); DVE the dy=+-1 rows
DVE_TAPS = [(-1, -1), (-1, 0), (-1, 1), (1, -1), (1, 0), (1, 1)]


def _build_nc(reps=1):
    import concourse.bass as bass
    import concourse.tile as tile
    from concourse import bacc, mybir
    from concourse.masks import make_identity

    f32 = mybir.dt.float32
    bf16 = mybir.dt.bfloat16
    Relu = mybir.ActivationFunctionType.Relu
    Gelu = mybir.ActivationFunctionType.Gelu
    Copy = mybir.ActivationFunctionType.Copy
    Ident = mybir.ActivationFunctionType.Identity
    add = mybir.AluOpType.add
    mult = mybir.AluOpType.mult

    nc = bacc.Bacc(None, target_bir_lowering=False)

    x_ext = nc.declare_dram_parameter("x", [C, HH, WW], bf16, isOutput=False)
    wqkv_ext = nc.declare_dram_parameter("wqkv", [C, 3 * C], bf16, isOutput=False)
    wproj_ext = nc.declare_dram_parameter("wproj", [C, C], bf16, isOutput=False)
    wfc1_ext = nc.declare_dram_parameter("wfc1", [C, CM], bf16, isOutput=False)
    wfc2_ext = nc.declare_dram_parameter("wfc2", [CM, C], bf16, isOutput=False)
    dww_ext = nc.declare_dram_parameter("dww", [C, 9], f32, isOutput=False)
    # pcst columns: 0=bq 1=dwb 2=A1A2 3=u12b 4=A2 5=invA2 6=negB2oA2 7=pad
    pcst_ext = nc.declare_dram_parameter("pcst", [C, 8], f32, isOutput=False)
    bkv_ext = nc.declare_dram_parameter("bkv", [2 * C], f32, isOutput=False)
    bfc1_ext = nc.declare_dram_parameter("bfc1c", [P, MH], f32, isOutput=False)
    out_ext = nc.declare_dram_parameter("out", [C, HH, WW], f32, isOutput=True)

    def bcast(ap_1d, parts=P):
        """[n] dram AP -> [parts, n] AP with 0-stride partition dim."""
        return bass.AP(
            tensor=ap_1d.tensor,
            offset=ap_1d.offset,
            ap=[[0, parts]] + list(ap_1d.ap),
        )

    with tile.TileContext(nc) as tc:
        with (
            tc.tile_pool(name="persist", bufs=1) as persist,
            tc.tile_pool(name="small", bufs=1) as small,
            tc.tile_pool(name="bigbf", bufs=6) as bigbf,
            tc.tile_pool(name="kvpool", bufs=4) as kvpool,
            tc.tile_pool(name="hpool", bufs=3) as hpool,
            tc.tile_pool(name="dwsb", bufs=6) as dwsb_pool,
            tc.tile_pool(name="outsb", bufs=2) as outsb_pool,
            tc.tile_pool(name="psA", bufs=4, space="PSUM") as psA,
            tc.tile_pool(name="psDw", bufs=2, space="PSUM") as psDw,
            tc.tile_pool(name="psHold", bufs=2, space="PSUM") as psHold,
        ):
            for rep in range(reps):
                # ------------- load x (bf16, cast host-side) --------------
                x2d = x_ext[:].rearrange("c h w -> c (h w)")
                xb = [bigbf.tile([P, N], bf16, tag="bigbf", name="xb")
                      for _ in range(2)]
                wqkv = [persist.tile([P, 3 * C], bf16, tag=f"wqkv_{k}",
                                     name="wqkv") for k in range(2)]
                wproj = [persist.tile([P, C], bf16, tag=f"wproj_{k}",
                                      name="wproj") for k in range(2)]
                wfc1 = [persist.tile([P, CM], bf16, tag=f"wfc1_{k}",
                                     name="wfc1") for k in range(2)]
                wfc2 = persist.tile([P, MH, C], bf16, tag="wfc2", name="wfc2")
                # ------------- small constants ----------------------------
                pcst = [small.tile([P, 8], f32, tag=f"pcst_{m}", name="pcst")
                        for m in range(2)]
                for m in range(2):
                    nc.gpsimd.dma_start(
                        out=pcst[m][:], in_=pcst_ext[m * P:(m + 1) * P, :]
                    )
                bq = [pcst[m][:, 0:1] for m in range(2)]
                dwb = [pcst[m][:, 1:2] for m in range(2)]
                A1A2 = [pcst[m][:, 2:3] for m in range(2)]
                u12b = [pcst[m][:, 3:4] for m in range(2)]
                A2 = [pcst[m][:, 4:5] for m in range(2)]
                invA2 = [pcst[m][:, 5:6] for m in range(2)]
                negB2oA2 = [pcst[m][:, 6:7] for m in range(2)]

                bias_kv = small.tile([P, 2 * C], f32, tag="bias_kv",
                                     name="bias_kv")
                nc.gpsimd.dma_start(out=bias_kv[:], in_=bcast(bkv_ext[:]))

                bfc1_sb = small.tile([P, MH], f32, tag="bfc1_sb",
                                     name="bfc1_sb")
                nc.gpsimd.dma_start(out=bfc1_sb[:], in_=bfc1_ext[:])

                dma_engs = [nc.sync, nc.scalar, nc.sync, nc.scalar]
                for k in range(2):
                    dma_engs[k].dma_start(
                        out=wqkv[k][:], in_=wqkv_ext[k * P:(k + 1) * P, :]
                    )
                for j in range(4):
                    s = slice(j * 1024, (j + 1) * 1024)
                    for m in range(2):
                        rows = slice(m * P, (m + 1) * P)
                        dma_engs[(2 * j + m) % 4].dma_start(
                            out=xb[m][:, s], in_=x2d[rows, s]
                        )
                for k in range(2):
                    rows = slice(k * P, (k + 1) * P)
                    nc.gpsimd.dma_start(out=wproj[k][:], in_=wproj_ext[rows, :])
                    dma_engs[k].dma_start(out=wfc1[k][:], in_=wfc1_ext[rows, :])
                for k in range(4):
                    dma_engs[k % 2].dma_start(
                        out=wfc2[:, 4 * k:4 * (k + 1), :],
                        in_=wfc2_ext[:].rearrange("(kt p) c -> p kt c", p=P)[
                            :, 4 * k:4 * (k + 1), :
                        ],
                    )


                # kv allocated first so v_sb (phase 3) reuses its slots
                kv_sb = [kvpool.tile([P, TT // 2, 2 * C], bf16, tag="kvpool",
                                     name="kv_sb") for _ in range(2)]
                u12 = [kvpool.tile([P, N], f32, tag="kvpool", name="u12")
                       for _ in range(2)]
                for m in range(2):
                    for j in range(4):
                        s = slice(j * 1024, (j + 1) * 1024)
                        nc.scalar.activation(
                            u12[m][:, s], xb[m][:, s], Ident,
                            bias=u12b[m], scale=A1A2[m],
                        )
                # ------------- phase 1: k,v then q then ctx ---------------
                q_sb = [bigbf.tile([P, N], bf16, tag="bigbf", name="q_sb")
                        for _ in range(2)]
                ctx_ps = [psDw.tile([P, NTC], f32, tag="psDw",
                                    name="ctx_ps") for _ in range(2)]

                def kv_step(tt):
                    ti, j = divmod(tt, TT // 2)
                    tcols = slice(tt * P, (tt + 1) * P)
                    ps = psA.tile([P, 2 * C], f32, tag="psA", name="kv_ps")
                    for k in range(2):
                        nc.tensor.matmul(
                            ps[:],
                            xb[k][:, tcols],
                            wqkv[k][:, C:3 * C],
                            start=(k == 0),
                            stop=(k == 1),
                        )
                    sl = kv_sb[ti][:, j, :]
                    nc.vector.tensor_tensor(sl, ps[:], bias_kv[:], add)
                    nc.scalar.activation(
                        kv_sb[ti][:, j, 0:C], kv_sb[ti][:, j, 0:C], Relu,
                        bias=0.0, scale=1.0,
                    )

                def ctx_step(tt):
                    ti, j = divmod(tt, TT // 2)
                    for m in range(2):
                        nc.tensor.matmul(
                            ctx_ps[m][:, 0:P],
                            kv_sb[ti][:, j, m * P:(m + 1) * P],
                            kv_sb[ti][:, j, C + m * P:C + (m + 1) * P],
                            start=(tt == 0),
                            stop=(tt == TT - 1),
                        )

                def q_step(nt):
                    cols = slice(nt * NTC, (nt + 1) * NTC)
                    for m in range(2):
                        ps = psA.tile([P, NTC], f32, tag="psA", name="q_ps")
                        for k in range(2):
                            nc.tensor.matmul(
                                ps[:],
                                wqkv[k][:, m * P:(m + 1) * P],
                                xb[k][:, cols],
                                start=(k == 0),
                                stop=(k == 1),
                            )
                        nc.scalar.activation(
                            q_sb[m][:, cols], ps[:], Relu, bias=bq[m],
                            scale=1.0,
                        )

                for tt in range(TT):
                    kv_step(tt)
                    if tt % 4 == 3:
                        q_step(tt // 4)

                ident = small.tile([P, P], bf16, tag="ident", name="ident")
                make_identity(nc, ident[:])
                dwwt = []
                diag_c = []
                for m in range(2):
                    t = small.tile([P, 9], f32, tag=f"dww_{m}", name="dwwt")
                    nc.gpsimd.dma_start(
                        out=t[:], in_=dww_ext[m * P:(m + 1) * P, :]
                    )
                    dwwt.append(t)
                    d = small.tile([P, 9, P], bf16, tag=f"diagc_{m}",
                                   name="diagc")
                    for i in range(9):
                        nc.vector.tensor_scalar_mul(
                            d[:, i, :], ident[:], t[:, i:i + 1]
                        )
                    diag_c.append(d)
                for tt in range(TT):
                    ctx_step(tt)


                # extract per-head diag blocks (scaled) to bf16
                ctx_sb = [small.tile([P, HD], bf16, tag=f"ctx_{m}", name="ctx")
                          for m in range(2)]
                for h in range(NH):
                    m, r = divmod(h, 4)
                    rows = slice(32 * r, 32 * r + 32)
                    nc.scalar.activation(
                        ctx_sb[m][rows, :],
                        ctx_ps[m][rows, 32 * r:32 * r + 32],
                        Copy,
                        scale=SCALE,
                    )

                # ------------- phase 2: attn out (feature-major) ----------
                attn_sb = [bigbf.tile([P, N], bf16, tag="bigbf", name="attn")
                           for _ in range(2)]

                def attn_step(nt):
                    cols = slice(nt * NTC, (nt + 1) * NTC)
                    for m in range(2):
                        ps = psA.tile([P, NTC], f32, tag="psA", name="attn_ps")
                        for j in range(4):
                            rows = slice(32 * j, 32 * j + 32)
                            nc.tensor.matmul(
                                ps[rows, :],
                                ctx_sb[m][rows, :],
                                q_sb[m][rows, cols],
                                start=True,
                                stop=True,
                                tile_position=(32 * j, 32 * j),
                            )
                        nc.scalar.activation(
                            attn_sb[m][:, cols], ps[:], Copy, scale=1.0
                        )

                # ------------- phases 3+4 interleaved per n-tile ----------
                v_sb = [kvpool.tile([P, N], f32, tag="kvpool", name="v_sb")
                        for _ in range(2)]
                t1_b = [bigbf.tile([P, N], bf16, tag="bigbf", name="t1_b")
                        for _ in range(2)]

                def clip(dy, dx, y0):
                    ys = max(y0, -dy)
                    ye = min(y0 + YB, HH - dy)
                    xs = max(0, -dx)
                    xe = min(WW, WW - dx)
                    return ys, ye, xs, xe

                dw_tiles = {}

                def phase3a(nt, pe_all=False):
                    y0 = nt * YB
                    dw_chunks = []
                    for m in range(2):
                        av = attn_sb[m][:].rearrange("p (y x) -> p y x", x=WW)
                        ps = psDw.tile([P, YB, WW], f32, tag="psDw",
                                       name="dw_ps")
                        nc.tensor.matmul(
                            ps[:], diag_c[m][:, 4, :], av[:, y0:y0 + YB, :],
                            start=True, stop=False, skip_group_check=True,
                        )
                        if pe_all:
                            for i, (dy, dx) in enumerate(DVE_TAPS):
                                ys, ye, xs, xe = clip(dy, dx, y0)
                                ti = (dy + 1) * 3 + (dx + 1)
                                nc.tensor.matmul(
                                    ps[:, ys - y0:ye - y0, xs:xe],
                                    diag_c[m][:, ti, :],
                                    av[:, ys + dy:ye + dy, xs + dx:xe + dx],
                                    start=False, stop=False,
                                    skip_group_check=True,
                                )
                        nc.tensor.matmul(
                            ps[:, :, 1:WW], diag_c[m][:, 3, :],
                            av[:, y0:y0 + YB, 0:WW - 1],
                            start=False, stop=False, skip_group_check=True,
                        )
                        nc.tensor.matmul(
                            ps[:, :, 0:WW - 1], diag_c[m][:, 5, :],
                            av[:, y0:y0 + YB, 1:WW],
                            start=False, stop=True, skip_group_check=True,
                        )
                        dve_taps = [] if pe_all else DVE_TAPS
                        for (dy, dx) in dve_taps:
                            ys, ye, xs, xe = clip(dy, dx, y0)
                            ti = (dy + 1) * 3 + (dx + 1)
                            ym = (ys + ye) // 2
                            for (ya, yb_) in ((ys, ym), (ym, ye)):
                                if ya >= yb_:
                                    continue
                                nc.vector.scalar_tensor_tensor(
                                    ps[:, ya - y0:yb_ - y0, xs:xe],
                                    av[:, ya + dy:yb_ + dy, xs + dx:xe + dx],
                                    dwwt[m][:, ti:ti + 1],
                                    ps[:, ya - y0:yb_ - y0, xs:xe],
                                    mult, add,
                                )
                        dwc = dwsb_pool.tile([P, NTC], bf16, tag="dwsb",
                                             name="dwc")
                        nc.vector.tensor_scalar(
                            dwc[:].rearrange("p (y x) -> p y x", x=WW),
                            ps[:], dwb[m], None, add,
                        )
                        dw_chunks.append(dwc)
                    dw_tiles[nt] = dw_chunks

                def phase3b(nt):
                    cols = slice(nt * NTC, (nt + 1) * NTC)
                    dw_chunks = dw_tiles.pop(nt)
                    for mo in range(2):
                        ps = psA.tile([P, NTC], f32, tag="psA", name="proj_ps")
                        for k in range(2):
                            nc.tensor.matmul(
                                ps[:],
                                wproj[k][:, mo * P:(mo + 1) * P],
                                dw_chunks[k][:],
                                start=(k == 0),
                                stop=(k == 1),
                            )
                        nc.vector.scalar_tensor_tensor(
                            v_sb[mo][:, cols], ps[:], A1A2[mo],
                            u12[mo][:, cols], mult, add,
                        )
                        nc.vector.tensor_scalar(
                            t1_b[mo][:, cols], v_sb[mo][:, cols],
                            invA2[mo], negB2oA2[mo], mult, add,
                        )

                def phase4(nt):
                    cols = slice(nt * NTC, (nt + 1) * NTC)
                    h_sb = hpool.tile([P, MH, NTC], bf16, tag="hpool",
                                      name="h_sb")
                    fc2_ps = [psHold.tile([P, NTC], f32, tag="psHold",
                                          name="fc2_ps") for _ in range(2)]
                    for kt in range(MH):
                        ps = psA.tile([P, NTC], f32, tag="psA", name="fc1_ps")
                        for k in range(2):
                            nc.tensor.matmul(
                                ps[:],
                                wfc1[k][:, kt * P:(kt + 1) * P],
                                t1_b[k][:, cols],
                                start=(k == 0),
                                stop=(k == 1),
                            )
                        nc.scalar.activation(
                            h_sb[:, kt, :], ps[:], Gelu,
                            bias=bfc1_sb[:, kt:kt + 1], scale=1.0,
                        )
                        for mo in range(2):
                            nc.tensor.matmul(
                                fc2_ps[mo][:],
                                wfc2[:, kt, mo * P:(mo + 1) * P],
                                h_sb[:, kt, :],
                                start=(kt == 0),
                                stop=(kt == MH - 1),
                            )
                    for mo in range(2):
                        ot = outsb_pool.tile([P, NTC], f32, tag="outsb",
                                             name="outsb")
                        nc.vector.scalar_tensor_tensor(
                            ot[:], fc2_ps[mo][:], A2[mo], v_sb[mo][:, cols],
                            mult, add,
                        )
                        nc.sync.dma_start(
                            out=out_ext[:].rearrange("c h w -> c (h w)")[
                                mo * P:(mo + 1) * P, cols
                            ],
                            in_=ot[:],
                        )

                attn_step(0)
                attn_step(1)
                attn_step(2)
                phase3a(0, pe_all=True)
                for nt in range(NT):
                    if nt + 3 < NT:
                        attn_step(nt + 3)
                    phase3b(nt)
                    if nt + 1 < NT:
                        phase3a(nt + 1, pe_all=(nt == 0))
                    if nt >= 1:
                        phase4(nt - 1)
                phase4(NT - 1)

    nc.compile()
    return nc


def _get_nc(reps=1):
    key = ("nc", reps)
    if key not in _CACHE:
        _CACHE[key] = _build_nc(reps)
    return _CACHE[key]


def _prep_shared(inputs):
    import ml_dtypes

    bf = ml_dtypes.bfloat16
    f = lambda k: np.asarray(inputs[k], dtype=np.float32)

    rs1 = 1.0 / np.sqrt(f("bn1_v") + EPS)
    gr1 = f("bn1_g") * rs1
    A1 = gr1 + f("alpha1")
    B1 = f("bn1_b") - f("bn1_m") * gr1 + A1 * f("bproj")
    rs2 = 1.0 / np.sqrt(f("bn2_v") + EPS)
    gr2 = f("bn2_g") * rs2
    A2 = gr2 + f("alpha2")
    B2 = f("bn2_b") - f("bn2_m") * gr2 + A2 * f("bfc2")

    A1A2 = A1 * A2
    u12b = A2 * B1 + B2
    invA2 = 1.0 / A2
    negB2oA2 = -B2 / A2
    pad = np.zeros_like(A2)

    pcst = np.stack(
        [f("bqkv")[:C], f("dw_b"), A1A2, u12b, A2, invA2, negB2oA2, pad],
        axis=1,
    )

    return {
        "wqkv": np.ascontiguousarray(f("Wqkv").astype(bf)),
        "wproj": np.ascontiguousarray(f("Wproj").astype(bf)),
        "wfc1": np.ascontiguousarray(f("Wfc1").astype(bf)),
        "wfc2": np.ascontiguousarray(f("Wfc2").astype(bf)),
        "dww": np.ascontiguousarray(f("dw_w").reshape(C, 9)),
        "pcst": np.ascontiguousarray(pcst.astype(np.float32)),
        "bkv": np.ascontiguousarray(f("bqkv")[C:]),
        "bfc1c": np.ascontiguousarray(f("bfc1").reshape(MH, P).T),
    }


def kernel(**inputs):
    import ml_dtypes

    from concourse.bass_utils import run_bass_kernel_spmd

    nc = _get_nc()
    shared = _prep_shared(inputs)
    x = np.ascontiguousarray(
        np.asarray(inputs["x"], dtype=np.float32).astype(ml_dtypes.bfloat16)
    )
    in_maps = [dict(shared, x=x[i]) for i in range(B)]
    res = run_bass_kernel_spmd(nc, in_maps, core_ids=list(range(B)))
    return np.stack([res.results[i]["out"] for i in range(B)], axis=0)


def make_in_maps(inputs):
    import ml_dtypes

    shared = _prep_shared(inputs)
    x = np.ascontiguousarray(
        np.asarray(inputs["x"], dtype=np.float32).astype(ml_dtypes.bfloat16)
    )
    return [dict(shared, x=x[i]) for i in range(B)]
